# revision 34
# baseline (speedup 1.0000x reference)
"""Trainium2 Bass kernel for the recurrent spiking NN (RSNN) problem.

Strategy (data-parallel over batch, per sharding hint):
  - 128 batch rows -> 16 per core x 8 cores; fc1/recurrent replicated.
  - All on-chip state kept TRANSPOSED: hidden dim H=1024 split into 8
    chunks of 128 partitions, batch (16) on the free dim. This keeps
    every vector op at full 128-partition utilization and avoids any
    per-step transposes.
  - Phase 1: xcur[h, (t,b)] = sum_i fc1[i,h] * x[b,t,i] precomputed with
    big matmuls (fc1 stationary, host-pre-transposed xT moving), written
    to a DRAM intermediate (32MB/core), streamed back during the scan.
  - Phase 2: 500 sequential steps; per step 64 matmuls (recurrent
    [128,128] bf16 stationary tiles, spikes [128,16] bf16 moving) with
    fp32 PSUM accumulation, then fused vector ops for the LIF update:
        nm   = decay * mr + xc_t          (scalar_tensor_tensor)
        mem  = nm + cur_psum              (tensor_tensor)
        spk  = mem >= 1.0    -> bf16      (tensor_scalar, is_ge)
        mr   = (mem < 1.0) * mem          (scalar_tensor_tensor)
    Output slice spk[:102] accumulated in SBUF, one DMA at the end.
  - bf16 for all matmul operands is safe: empirically the network is
    saturated (spike density ~0.994, threshold margins >> bf16 noise);
    casting x/fc1/recurrent to bf16 reproduces the fp32 reference
    exactly on CPU. Accumulation stays fp32 (PSUM) and the LIF state
    stays fp32.

kernel(**inputs) takes FULL inputs and returns the FULL output, matching
reference.reference()'s return structure: (out[128,500,102] f32, T).
"""

import os

import numpy as np
import ml_dtypes

# Problem constants (hardcoded per contest rules -- no spec.json reads).
B_TOTAL = 128
T_STEPS = int(os.environ.get("RSNN_T_OVERRIDE", "500"))
N_IN = 784
H = 1024
N_CORES = 8
BL = B_TOTAL // N_CORES          # 16 batch rows per core
NS = H // 10                     # 102 output hidden slice
V_TH = 1.0
DECAY = float(1.0 / (1.0 + np.exp(-2.0)))   # sigmoid(INIT_TAU=2.0)

KI, NKI = 128, 6                 # 784 = 6 x 128 + 16 input-contraction chunks
KTAIL = N_IN - KI * NKI          # 16-row tail chunk
MC = 8                           # 1024 = 8 x 128 hidden chunks (output side)
JC = 8                           # 1024 = 8 x 128 hidden chunks (contraction)
NCOLS = T_STEPS * BL             # moving columns in phase 1
PH1_NB = 512                     # phase-1 moving tile (columns per matmul)
CHUNK_T = 32                     # scan steps per xcur ring-buffer refill

_CACHE: dict = {}


def _build_program():
    import concourse.bass as bass
    import concourse.bacc as bacc
    import concourse.mybir as mybir
    from concourse import tile
    from concourse.tile_rust import add_dep_helper

    f32 = mybir.dt.float32
    bf16 = mybir.dt.bfloat16
    Alu = mybir.AluOpType

    nc = bacc.Bacc(
        "TRN2",
        target_bir_lowering=False,
        debug=False,
        enable_asserts=False,
        num_devices=N_CORES,
    )

    xT = nc.dram_tensor("xT", [N_IN, NCOLS], bf16, kind="ExternalInput")
    fc1 = nc.dram_tensor("fc1", [N_IN, H], bf16, kind="ExternalInput")
    rec = nc.dram_tensor("rec", [H, H], bf16, kind="ExternalInput")
    out = nc.dram_tensor("out", [NS, NCOLS], f32, kind="ExternalOutput")

    n_ph1 = (NCOLS + PH1_NB - 1) // PH1_NB  # phase-1 column blocks (incl. tail)
    n_tb = (T_STEPS + CHUNK_T - 1) // CHUNK_T

    with tile.TileContext(nc) as tc:
        with (
            tc.tile_pool(name="wpool", bufs=1) as wpool,
            tc.tile_pool(name="xpool", bufs=2) as xpool,
            tc.tile_pool(name="ph1ps", bufs=2, space="PSUM") as ph1ps,
            tc.tile_pool(name="dram", bufs=1, space="DRAM") as dpool,
            tc.tile_pool(name="spool", bufs=1) as spool,
            tc.tile_pool(name="ring", bufs=2) as ring,
            tc.tile_pool(name="scps", bufs=2, space="PSUM") as scps,
        ):
            # ---- load replicated weights ----
            # fc1 in 6 x K=128 chunks (FWL-eligible) + one K=16 tail
            fc1_sb = wpool.tile([KI, NKI, H], bf16)
            nc.sync.dma_start(
                fc1_sb[:],
                fc1.ap()[0:KI * NKI, :].rearrange("(c p) h -> p c h", p=KI),
            )
            fc1_tl = wpool.tile([KTAIL, H], bf16)
            nc.sync.dma_start(fc1_tl[:], fc1.ap()[KI * NKI:, :])
            rec_sb = wpool.tile([128, JC, H], bf16)
            nc.sync.dma_start(
                rec_sb[:], rec.ap().rearrange("(c p) h -> p c h", p=128)
            )

            # DRAM intermediate holding xcur, transposed: [mchunk, part, (t,b)]
            xcT = dpool.tile([MC, 128, NCOLS], f32)

            # ---- phase 1: xcur = x @ fc1 (transposed output) ----
            # Emitted as a generator yielding after each PE matmul so the
            # work can be paced into the scan's per-step PE idle gaps
            # instead of jamming the first ~50 steps.
            ph1_stores = [[None] * MC for _ in range(n_ph1)]

            def ph1_gen():
                for n in range(n_ph1):
                    c0 = n * PH1_NB
                    cn = min(PH1_NB, NCOLS - c0)
                    xt_sb = xpool.tile([KI, NKI, PH1_NB], bf16, tag="xt",
                                       name="xt_sb")
                    nc.sync.dma_start(
                        xt_sb[:, :, :cn],
                        xT.ap()[0:KI * NKI, c0:c0 + cn].rearrange(
                            "(c p) n -> p c n", p=KI
                        ),
                    )
                    xt_tl = xpool.tile([KTAIL, PH1_NB], bf16, tag="xttl",
                                       name="xt_tl")
                    nc.sync.dma_start(
                        xt_tl[:, :cn], xT.ap()[KI * NKI:, c0:c0 + cn]
                    )
                    for m in range(MC):
                        ps = ph1ps.tile([128, PH1_NB], f32, tag="ph1",
                                        name="ph1ps")
                        # K=16 tail first: its non-FWL weight load hides
                        # under subsequent full-chunk streams
                        mm = nc.tensor.matmul(
                            ps[:, :cn],
                            fc1_tl[:, m * 128:(m + 1) * 128],
                            xt_tl[:, :cn],
                            start=True,
                            stop=False,
                        )
                        yield mm
                        for k in range(NKI):
                            mm = nc.tensor.matmul(
                                ps[:, :cn],
                                fc1_sb[:, k, m * 128:(m + 1) * 128],
                                xt_sb[:, k, :cn],
                                start=False,
                                stop=(k == NKI - 1),
                            )
                            yield mm
                        evac = xpool.tile([128, PH1_NB], f32, tag="evac",
                                          name="evac")
                        # store xc - 1 (Scalar engine; spike test needs no +1)
                        nc.scalar.activation(
                            evac[:, :cn], ps[:, :cn],
                            mybir.ActivationFunctionType.Copy, bias=-1.0,
                        )
                        st = nc.sync.dma_start(
                            xcT[m, :, c0:c0 + cn], evac[:, :cn]
                        )
                        ph1_stores[n][m] = st

            _ph1 = ph1_gen()

            def emit_ph1(k):
                """Emit up to k phase-1 matmuls; return their instructions."""
                mms = []
                for _ in range(k):
                    mm = next(_ph1, None)
                    if mm is None:
                        break
                    mms.append(mm)
                return mms

            # prologue: two column blocks so the scan can start; the rest
            # is emitted pinned into per-step PE idle windows below
            emit_ph1(2 * MC * (NKI + 1))

            # ---- phase 2: sequential LIF scan ----
            spk = [spool.tile([128, JC, BL], bf16, tag=f"spk{i}", name=f"spk{i}") for i in range(2)]
            mr = [spool.tile([128, JC, BL], f32, tag=f"mr{i}", name=f"mr{i}") for i in range(2)]
            nm = [spool.tile([128, JC, BL], f32, tag=f"nm{i}", name=f"nm{i}") for i in range(2)]
            mem = [spool.tile([128, JC, BL], f32, tag=f"mem{i}", name=f"mem{i}") for i in range(2)]
            outb = spool.tile([NS, T_STEPS, BL], f32, tag="outb")

            nc.vector.memset(spk[1][:], 0.0)
            nc.vector.memset(mr[1][:], 0.0)

            pend_ph1 = []

            for tb in range(n_tb):
                t0 = tb * CHUNK_T
                tn = min(CHUNK_T, T_STEPS - t0)
                xc_sb = ring.tile([128, MC, CHUNK_T * BL], f32, tag="xcring")
                # per-m-chunk DMAs: APs mirror the phase-1 store slices so
                # the DRAM dependency tracker reliably orders read-after-write
                for m in range(MC):
                    pref = nc.sync.dma_start(
                        xc_sb[:, m, :tn * BL],
                        xcT[m, :, t0 * BL:(t0 + tn) * BL],
                    )
                    # DRAM-tile RAW tracking misses these; order explicitly
                    st = ph1_stores[tb][m]
                    if st is not None:
                        add_dep_helper(
                            pref.ins, st.ins, reason="xc ring after ph1 store"
                        )
                for tt in range(tn):
                    t = t0 + tt
                    p, q = t % 2, (t + 1) % 2
                    # Three PSUM banks by output chunk. The matmul stream is
                    # emitted j-block-major so the NEXT step's stream (which
                    # consumes spike chunks in the same block order) never
                    # stalls: its first matmuls need only chunks 0-1, which
                    # the tiny first compare publishes right after this
                    # stream's end-of-stream semaphore.
                    SPLITS = ((0, 2), (2, 5), (5, 8))
                    pss = [
                        scps.tile([128, hi - lo, BL], f32, tag=f"scan{i}",
                                  name=f"scanps{i}")
                        for i, (lo, hi) in enumerate(SPLITS)
                    ]

                    def bank(m):
                        i = 0 if m < 2 else (1 if m < 5 else 2)
                        return pss[i][:, m - SPLITS[i][0], :]

                    xct = xc_sb[:, :, tt * BL:(tt + 1) * BL]  # holds xc - 1
                    # neg = -decay*mr_prev - (xc-1) = 1 - decay*mr - xc
                    # (runs during the matmuls; spike test becomes ps >= neg)
                    nc.vector.scalar_tensor_tensor(
                        nm[p][:], mr[q][:], -DECAY, xct, Alu.mult, Alu.subtract
                    )
                    # one accumulation group per bank: start on the bank's
                    # first matmul (clears has_written for the whole bank;
                    # later first-writes to other offsets overwrite), stop
                    # on its last
                    first_mm = None
                    last_mm = None
                    for jlo, jhi in SPLITS:      # j-blocks
                        for m in range(MC):
                            dst = bank(m)
                            blo, bhi = SPLITS[0 if m < 2 else (1 if m < 5 else 2)]
                            for j in range(jlo, jhi):
                                mm = nc.tensor.matmul(
                                    dst,
                                    rec_sb[:, j, m * 128:(m + 1) * 128],
                                    spk[q][:, j, :],
                                    start=(j == 0 and m == blo),
                                    stop=(j == JC - 1 and m == bhi - 1),
                                )
                                if first_mm is None:
                                    first_mm = mm
                                last_mm = mm
                    # pin pending phase-1 matmuls into the PE idle window
                    # between the previous step's stream and this one
                    for pm in pend_ph1:
                        add_dep_helper(
                            first_mm.ins, pm.ins, sync=False,
                            reason="ph1 mm before next scan stream",
                        )
                    pend_ph1 = emit_ph1(2)
                    for pm in pend_ph1:
                        add_dep_helper(
                            pm.ins, last_mm.ins, sync=False,
                            reason="ph1 mm after scan stream",
                        )
                    # spk = (cur_rec >= neg), smallest chunk set first
                    for i, (lo, hi) in enumerate(SPLITS):
                        nc.vector.tensor_tensor(
                            spk[p][:, lo:hi, :],
                            pss[i][:],
                            nm[p][:, lo:hi, :],
                            Alu.is_ge,
                        )
                    # mem = (ps + 1) - neg   (off critical path)
                    for i, (lo, hi) in enumerate(SPLITS):
                        nc.vector.scalar_tensor_tensor(
                            mem[p][:, lo:hi, :], pss[i][:], 1.0,
                            nm[p][:, lo:hi, :], Alu.add, Alu.subtract,
                        )
                    # mr = (mem < 1.0) * mem  == mem * (1 - spk)
                    nc.vector.scalar_tensor_tensor(
                        mr[p][:], mem[p][:], V_TH, mem[p][:], Alu.is_lt, Alu.mult
                    )
                    # output slice: first 102 hidden units live in chunk 0
                    nc.any.tensor_copy(outb[:, t, :], spk[p][0:NS, 0, :])

            nc.sync.dma_start(
                out.ap().rearrange("h (t b) -> h t b", t=T_STEPS), outb[:]
            )

    nc.compile()
    return nc


def _get_program():
    if "nc" not in _CACHE:
        _CACHE["nc"] = _build_program()
    return _CACHE["nc"]


def kernel(x: np.ndarray, fc1: np.ndarray, recurrent: np.ndarray):
    from concourse.bass_utils import run_bass_kernel_spmd

    nc = _get_program()

    x = np.asarray(x, dtype=np.float32)
    fc1_b = np.ascontiguousarray(np.asarray(fc1, np.float32)).astype(
        ml_dtypes.bfloat16
    )
    rec_b = np.ascontiguousarray(np.asarray(recurrent, np.float32)).astype(
        ml_dtypes.bfloat16
    )

    in_maps = []
    for c in range(N_CORES):
        xs = x[c * BL:(c + 1) * BL, :T_STEPS]          # [16, T, 784]
        xT_c = np.ascontiguousarray(xs.transpose(2, 1, 0).reshape(N_IN, NCOLS))
        in_maps.append(
            {"xT": xT_c.astype(ml_dtypes.bfloat16), "fc1": fc1_b, "rec": rec_b}
        )

    res = run_bass_kernel_spmd(nc, in_maps, list(range(N_CORES)))

    full = np.empty((B_TOTAL, T_STEPS, NS), dtype=np.float32)
    for c in range(N_CORES):
        o = np.asarray(res.results[c]["out"], dtype=np.float32)  # [102, T*16]
        full[c * BL:(c + 1) * BL] = o.reshape(NS, T_STEPS, BL).transpose(2, 1, 0)
    return full, T_STEPS


# revision 35
# speedup vs baseline: 1.0143x; 1.0143x over previous
"""Trainium2 Bass kernel for the recurrent spiking NN (RSNN) problem.

Strategy (data-parallel over batch, per sharding hint):
  - 128 batch rows -> 16 per core x 8 cores; fc1/recurrent replicated.
  - All on-chip state kept TRANSPOSED: hidden dim H=1024 split into 8
    chunks of 128 partitions, batch (16) on the free dim. This keeps
    every vector op at full 128-partition utilization and avoids any
    per-step transposes.
  - Phase 1: xcur[h, (t,b)] = sum_i fc1[i,h] * x[b,t,i] precomputed with
    big matmuls (fc1 stationary, host-pre-transposed xT moving), written
    to a DRAM intermediate (32MB/core), streamed back during the scan.
  - Phase 2: 500 sequential steps; per step 64 matmuls (recurrent
    [128,128] bf16 stationary tiles, spikes [128,16] bf16 moving) with
    fp32 PSUM accumulation, then fused vector ops for the LIF update:
        nm   = decay * mr + xc_t          (scalar_tensor_tensor)
        mem  = nm + cur_psum              (tensor_tensor)
        spk  = mem >= 1.0    -> bf16      (tensor_scalar, is_ge)
        mr   = (mem < 1.0) * mem          (scalar_tensor_tensor)
    Output slice spk[:102] accumulated in SBUF, one DMA at the end.
  - bf16 for all matmul operands is safe: empirically the network is
    saturated (spike density ~0.994, threshold margins >> bf16 noise);
    casting x/fc1/recurrent to bf16 reproduces the fp32 reference
    exactly on CPU. Accumulation stays fp32 (PSUM) and the LIF state
    stays fp32.

kernel(**inputs) takes FULL inputs and returns the FULL output, matching
reference.reference()'s return structure: (out[128,500,102] f32, T).
"""

import os

import numpy as np
import ml_dtypes

# Problem constants (hardcoded per contest rules -- no spec.json reads).
B_TOTAL = 128
T_STEPS = int(os.environ.get("RSNN_T_OVERRIDE", "500"))
N_IN = 784
H = 1024
N_CORES = 8
BL = B_TOTAL // N_CORES          # 16 batch rows per core
NS = H // 10                     # 102 output hidden slice
V_TH = 1.0
DECAY = float(1.0 / (1.0 + np.exp(-2.0)))   # sigmoid(INIT_TAU=2.0)

KI, NKI = 112, 7                 # 784 = 7 x 112 input-contraction chunks
MC = 8                           # 1024 = 8 x 128 hidden chunks (output side)
JC = 8                           # 1024 = 8 x 128 hidden chunks (contraction)
NCOLS = T_STEPS * BL             # moving columns in phase 1
PH1_NB = 512                     # phase-1 moving tile (columns per matmul)
CHUNK_T = 32                     # scan steps per xcur ring-buffer refill

_CACHE: dict = {}


def _build_program():
    import concourse.bass as bass
    import concourse.bacc as bacc
    import concourse.mybir as mybir
    from concourse import tile
    from concourse.tile_rust import add_dep_helper

    f32 = mybir.dt.float32
    bf16 = mybir.dt.bfloat16
    Alu = mybir.AluOpType

    nc = bacc.Bacc(
        "TRN2",
        target_bir_lowering=False,
        debug=False,
        enable_asserts=False,
        num_devices=N_CORES,
    )

    xT = nc.dram_tensor("xT", [N_IN, NCOLS], bf16, kind="ExternalInput")
    fc1 = nc.dram_tensor("fc1", [N_IN, H], bf16, kind="ExternalInput")
    rec = nc.dram_tensor("rec", [H, H], bf16, kind="ExternalInput")
    out = nc.dram_tensor("out", [NS, NCOLS], f32, kind="ExternalOutput")

    n_ph1 = (NCOLS + PH1_NB - 1) // PH1_NB  # phase-1 column blocks (incl. tail)
    n_tb = (T_STEPS + CHUNK_T - 1) // CHUNK_T

    with tile.TileContext(nc) as tc:
        with (
            tc.tile_pool(name="wpool", bufs=1) as wpool,
            tc.tile_pool(name="xpool", bufs=2) as xpool,
            tc.tile_pool(name="ph1ps", bufs=2, space="PSUM") as ph1ps,
            tc.tile_pool(name="dram", bufs=1, space="DRAM") as dpool,
            tc.tile_pool(name="spool", bufs=1) as spool,
            tc.tile_pool(name="ring", bufs=2) as ring,
            tc.tile_pool(name="scps", bufs=2, space="PSUM") as scps,
        ):
            # ---- load replicated weights ----
            fc1_sb = wpool.tile([KI, NKI, H], bf16)
            nc.sync.dma_start(
                fc1_sb[:], fc1.ap().rearrange("(c p) h -> p c h", p=KI)
            )
            rec_sb = wpool.tile([128, JC, H], bf16)
            nc.sync.dma_start(
                rec_sb[:], rec.ap().rearrange("(c p) h -> p c h", p=128)
            )

            # DRAM intermediate holding xcur, transposed: [mchunk, part, (t,b)]
            xcT = dpool.tile([MC, 128, NCOLS], f32)

            # ---- phase 1: xcur = x @ fc1 (transposed output) ----
            # Emitted as a generator yielding after each PE matmul so the
            # work can be paced into the scan's per-step PE idle gaps
            # instead of jamming the first ~50 steps.
            ph1_stores = [[None] * MC for _ in range(n_ph1)]

            def ph1_gen():
                for n in range(n_ph1):
                    c0 = n * PH1_NB
                    cn = min(PH1_NB, NCOLS - c0)
                    xt_sb = xpool.tile([KI, NKI, PH1_NB], bf16, tag="xt",
                                       name="xt_sb")
                    nc.sync.dma_start(
                        xt_sb[:, :, :cn],
                        xT.ap()[:, c0:c0 + cn].rearrange(
                            "(c p) n -> p c n", p=KI
                        ),
                    )
                    for m in range(MC):
                        ps = ph1ps.tile([128, PH1_NB], f32, tag="ph1",
                                        name="ph1ps")
                        for k in range(NKI):
                            mm = nc.tensor.matmul(
                                ps[:, :cn],
                                fc1_sb[:, k, m * 128:(m + 1) * 128],
                                xt_sb[:, k, :cn],
                                start=(k == 0),
                                stop=(k == NKI - 1),
                            )
                            yield mm
                        evac = xpool.tile([128, PH1_NB], f32, tag="evac",
                                          name="evac")
                        # store xc - 1 (Scalar engine; spike test needs no +1)
                        nc.scalar.activation(
                            evac[:, :cn], ps[:, :cn],
                            mybir.ActivationFunctionType.Copy, bias=-1.0,
                        )
                        st = nc.sync.dma_start(
                            xcT[m, :, c0:c0 + cn], evac[:, :cn]
                        )
                        ph1_stores[n][m] = st

            _ph1 = ph1_gen()

            def emit_ph1(k):
                """Emit up to k phase-1 matmuls; return their instructions."""
                mms = []
                for _ in range(k):
                    mm = next(_ph1, None)
                    if mm is None:
                        break
                    mms.append(mm)
                return mms

            # prologue: two column blocks so the scan can start; the rest
            # is emitted pinned into per-step PE idle windows below
            emit_ph1(2 * MC * NKI)

            # ---- phase 2: sequential LIF scan ----
            spk = [spool.tile([128, JC, BL], bf16, tag=f"spk{i}", name=f"spk{i}") for i in range(2)]
            mr = [spool.tile([128, JC, BL], f32, tag=f"mr{i}", name=f"mr{i}") for i in range(2)]
            nm = [spool.tile([128, JC, BL], f32, tag=f"nm{i}", name=f"nm{i}") for i in range(2)]
            mem = [spool.tile([128, JC, BL], f32, tag=f"mem{i}", name=f"mem{i}") for i in range(2)]
            outb = spool.tile([NS, T_STEPS, BL], f32, tag="outb")

            nc.vector.memset(spk[1][:], 0.0)
            nc.vector.memset(mr[1][:], 0.0)

            pend_ph1 = []

            for tb in range(n_tb):
                t0 = tb * CHUNK_T
                tn = min(CHUNK_T, T_STEPS - t0)
                xc_sb = ring.tile([128, MC, CHUNK_T * BL], f32, tag="xcring")
                # per-m-chunk DMAs: APs mirror the phase-1 store slices so
                # the DRAM dependency tracker reliably orders read-after-write
                for m in range(MC):
                    pref = nc.sync.dma_start(
                        xc_sb[:, m, :tn * BL],
                        xcT[m, :, t0 * BL:(t0 + tn) * BL],
                    )
                    # DRAM-tile RAW tracking misses these; order explicitly
                    st = ph1_stores[tb][m]
                    if st is not None:
                        add_dep_helper(
                            pref.ins, st.ins, reason="xc ring after ph1 store"
                        )
                for tt in range(tn):
                    t = t0 + tt
                    p, q = t % 2, (t + 1) % 2
                    # Three PSUM banks by output chunk. The matmul stream is
                    # emitted j-block-major so the NEXT step's stream (which
                    # consumes spike chunks in the same block order) never
                    # stalls: its first matmuls need only chunks 0-1, which
                    # the tiny first compare publishes right after this
                    # stream's end-of-stream semaphore.
                    SPLITS = ((0, 2), (2, 5), (5, 8))
                    pss = [
                        scps.tile([128, hi - lo, BL], f32, tag=f"scan{i}",
                                  name=f"scanps{i}")
                        for i, (lo, hi) in enumerate(SPLITS)
                    ]

                    def bank(m):
                        i = 0 if m < 2 else (1 if m < 5 else 2)
                        return pss[i][:, m - SPLITS[i][0], :]

                    xct = xc_sb[:, :, tt * BL:(tt + 1) * BL]  # holds xc - 1
                    # neg = -decay*mr_prev - (xc-1) = 1 - decay*mr - xc
                    # (runs during the matmuls; spike test becomes ps >= neg)
                    nc.vector.scalar_tensor_tensor(
                        nm[p][:], mr[q][:], -DECAY, xct, Alu.mult, Alu.subtract
                    )
                    # one accumulation group per bank: start on the bank's
                    # first matmul (clears has_written for the whole bank;
                    # later first-writes to other offsets overwrite), stop
                    # on its last
                    first_mm = None
                    last_mm = None
                    for jlo, jhi in SPLITS:      # j-blocks
                        for m in range(MC):
                            dst = bank(m)
                            blo, bhi = SPLITS[0 if m < 2 else (1 if m < 5 else 2)]
                            for j in range(jlo, jhi):
                                mm = nc.tensor.matmul(
                                    dst,
                                    rec_sb[:, j, m * 128:(m + 1) * 128],
                                    spk[q][:, j, :],
                                    start=(j == 0 and m == blo),
                                    stop=(j == JC - 1 and m == bhi - 1),
                                )
                                if first_mm is None:
                                    first_mm = mm
                                last_mm = mm
                    # pin pending phase-1 matmuls into the PE idle window
                    # between the previous step's stream and this one
                    for pm in pend_ph1:
                        add_dep_helper(
                            first_mm.ins, pm.ins, sync=False,
                            reason="ph1 mm before next scan stream",
                        )
                    pend_ph1 = emit_ph1(2)
                    for pm in pend_ph1:
                        add_dep_helper(
                            pm.ins, last_mm.ins, sync=False,
                            reason="ph1 mm after scan stream",
                        )
                    # spk = (cur_rec >= neg), smallest chunk set first
                    for i, (lo, hi) in enumerate(SPLITS):
                        nc.vector.tensor_tensor(
                            spk[p][:, lo:hi, :],
                            pss[i][:],
                            nm[p][:, lo:hi, :],
                            Alu.is_ge,
                        )
                    # mem = (ps + 1) - neg   (off critical path)
                    for i, (lo, hi) in enumerate(SPLITS):
                        nc.vector.scalar_tensor_tensor(
                            mem[p][:, lo:hi, :], pss[i][:], 1.0,
                            nm[p][:, lo:hi, :], Alu.add, Alu.subtract,
                        )
                    # mr = (mem < 1.0) * mem  == mem * (1 - spk)
                    nc.vector.scalar_tensor_tensor(
                        mr[p][:], mem[p][:], V_TH, mem[p][:], Alu.is_lt, Alu.mult
                    )
                    # output slice: first 102 hidden units live in chunk 0
                    nc.any.tensor_copy(outb[:, t, :], spk[p][0:NS, 0, :])

            nc.sync.dma_start(
                out.ap().rearrange("h (t b) -> h t b", t=T_STEPS), outb[:]
            )

    nc.compile()
    return nc


def _get_program():
    if "nc" not in _CACHE:
        _CACHE["nc"] = _build_program()
    return _CACHE["nc"]


def kernel(x: np.ndarray, fc1: np.ndarray, recurrent: np.ndarray):
    from concourse.bass_utils import run_bass_kernel_spmd

    nc = _get_program()

    x = np.asarray(x, dtype=np.float32)
    fc1_b = np.ascontiguousarray(np.asarray(fc1, np.float32)).astype(
        ml_dtypes.bfloat16
    )
    rec_b = np.ascontiguousarray(np.asarray(recurrent, np.float32)).astype(
        ml_dtypes.bfloat16
    )

    in_maps = []
    for c in range(N_CORES):
        xs = x[c * BL:(c + 1) * BL, :T_STEPS]          # [16, T, 784]
        xT_c = np.ascontiguousarray(xs.transpose(2, 1, 0).reshape(N_IN, NCOLS))
        in_maps.append(
            {"xT": xT_c.astype(ml_dtypes.bfloat16), "fc1": fc1_b, "rec": rec_b}
        )

    res = run_bass_kernel_spmd(nc, in_maps, list(range(N_CORES)))

    full = np.empty((B_TOTAL, T_STEPS, NS), dtype=np.float32)
    for c in range(N_CORES):
        o = np.asarray(res.results[c]["out"], dtype=np.float32)  # [102, T*16]
        full[c * BL:(c + 1) * BL] = o.reshape(NS, T_STEPS, BL).transpose(2, 1, 0)
    return full, T_STEPS


# revision 36
# speedup vs baseline: 1.0152x; 1.0009x over previous
"""Trainium2 Bass kernel for the recurrent spiking NN (RSNN) problem.

Strategy (data-parallel over batch, per sharding hint):
  - 128 batch rows -> 16 per core x 8 cores; fc1/recurrent replicated.
  - All on-chip state kept TRANSPOSED: hidden dim H=1024 split into 8
    chunks of 128 partitions, batch (16) on the free dim. This keeps
    every vector op at full 128-partition utilization and avoids any
    per-step transposes.
  - Phase 1: xcur[h, (t,b)] = sum_i fc1[i,h] * x[b,t,i] precomputed with
    big matmuls (fc1 stationary, host-pre-transposed xT moving), written
    to a DRAM intermediate (32MB/core), streamed back during the scan.
  - Phase 2: 500 sequential steps; per step 64 matmuls (recurrent
    [128,128] bf16 stationary tiles, spikes [128,16] bf16 moving) with
    fp32 PSUM accumulation, then fused vector ops for the LIF update:
        nm   = decay * mr + xc_t          (scalar_tensor_tensor)
        mem  = nm + cur_psum              (tensor_tensor)
        spk  = mem >= 1.0    -> bf16      (tensor_scalar, is_ge)
        mr   = (mem < 1.0) * mem          (scalar_tensor_tensor)
    Output slice spk[:102] accumulated in SBUF, one DMA at the end.
  - bf16 for all matmul operands is safe: empirically the network is
    saturated (spike density ~0.994, threshold margins >> bf16 noise);
    casting x/fc1/recurrent to bf16 reproduces the fp32 reference
    exactly on CPU. Accumulation stays fp32 (PSUM) and the LIF state
    stays fp32.

kernel(**inputs) takes FULL inputs and returns the FULL output, matching
reference.reference()'s return structure: (out[128,500,102] f32, T).
"""

import os

import numpy as np
import ml_dtypes

# Problem constants (hardcoded per contest rules -- no spec.json reads).
B_TOTAL = 128
T_STEPS = int(os.environ.get("RSNN_T_OVERRIDE", "500"))
N_IN = 784
H = 1024
N_CORES = 8
BL = B_TOTAL // N_CORES          # 16 batch rows per core
NS = H // 10                     # 102 output hidden slice
V_TH = 1.0
DECAY = float(1.0 / (1.0 + np.exp(-2.0)))   # sigmoid(INIT_TAU=2.0)

KI, NKI = 112, 7                 # 784 = 7 x 112 input-contraction chunks
MC = 8                           # 1024 = 8 x 128 hidden chunks (output side)
JC = 8                           # 1024 = 8 x 128 hidden chunks (contraction)
NCOLS = T_STEPS * BL             # moving columns in phase 1
PH1_NB = 512                     # phase-1 moving tile (columns per matmul)
CHUNK_T = 32                     # scan steps per xcur ring-buffer refill

_CACHE: dict = {}


def _build_program():
    import concourse.bass as bass
    import concourse.bacc as bacc
    import concourse.mybir as mybir
    from concourse import tile
    from concourse.tile_rust import add_dep_helper

    f32 = mybir.dt.float32
    bf16 = mybir.dt.bfloat16
    Alu = mybir.AluOpType

    nc = bacc.Bacc(
        "TRN2",
        target_bir_lowering=False,
        debug=False,
        enable_asserts=False,
        num_devices=N_CORES,
    )

    xT = nc.dram_tensor("xT", [N_IN, NCOLS], bf16, kind="ExternalInput")
    fc1 = nc.dram_tensor("fc1", [N_IN, H], bf16, kind="ExternalInput")
    rec = nc.dram_tensor("rec", [H, H], bf16, kind="ExternalInput")
    out = nc.dram_tensor("out", [NS, NCOLS], f32, kind="ExternalOutput")

    n_ph1 = (NCOLS + PH1_NB - 1) // PH1_NB  # phase-1 column blocks (incl. tail)
    n_tb = (T_STEPS + CHUNK_T - 1) // CHUNK_T

    with tile.TileContext(nc) as tc:
        with (
            tc.tile_pool(name="wpool", bufs=1) as wpool,
            tc.tile_pool(name="xpool", bufs=2) as xpool,
            tc.tile_pool(name="ph1ps", bufs=2, space="PSUM") as ph1ps,
            tc.tile_pool(name="dram", bufs=1, space="DRAM") as dpool,
            tc.tile_pool(name="spool", bufs=1) as spool,
            tc.tile_pool(name="ring", bufs=2) as ring,
            tc.tile_pool(name="scps", bufs=2, space="PSUM") as scps,
        ):
            # ---- load replicated weights ----
            fc1_sb = wpool.tile([KI, NKI, H], bf16)
            nc.sync.dma_start(
                fc1_sb[:], fc1.ap().rearrange("(c p) h -> p c h", p=KI)
            )
            rec_sb = wpool.tile([128, JC, H], bf16)
            nc.sync.dma_start(
                rec_sb[:], rec.ap().rearrange("(c p) h -> p c h", p=128)
            )

            # DRAM intermediate holding xcur, transposed: [mchunk, part, (t,b)]
            xcT = dpool.tile([MC, 128, NCOLS], f32)

            # ---- phase 1: xcur = x @ fc1 (transposed output) ----
            # Emitted as a generator yielding after each PE matmul so the
            # work can be paced into the scan's per-step PE idle gaps
            # instead of jamming the first ~50 steps.
            ph1_stores = [[None] * MC for _ in range(n_ph1)]

            def ph1_gen():
                for n in range(n_ph1):
                    c0 = n * PH1_NB
                    cn = min(PH1_NB, NCOLS - c0)
                    xt_sb = xpool.tile([KI, NKI, PH1_NB], bf16, tag="xt",
                                       name="xt_sb")
                    nc.sync.dma_start(
                        xt_sb[:, :, :cn],
                        xT.ap()[:, c0:c0 + cn].rearrange(
                            "(c p) n -> p c n", p=KI
                        ),
                    )
                    for m in range(MC):
                        ps = ph1ps.tile([128, PH1_NB], f32, tag="ph1",
                                        name="ph1ps")
                        # N=256 halves: smaller quanta pack the scan's
                        # per-step PE idle window better, and the second
                        # half reuses the same stationary weights
                        halves = [(0, 256), (256, cn)] if cn > 256 else [(0, cn)]
                        for k in range(NKI):
                            for hi, (hl, hr) in enumerate(halves):
                                mm = nc.tensor.matmul(
                                    ps[:, hl:hr],
                                    fc1_sb[:, k, m * 128:(m + 1) * 128],
                                    xt_sb[:, k, hl:hr],
                                    start=(k == 0 and hi == 0),
                                    stop=(k == NKI - 1 and hi == len(halves) - 1),
                                )
                                yield mm
                        evac = xpool.tile([128, PH1_NB], f32, tag="evac",
                                          name="evac")
                        # store xc - 1 (Scalar engine; spike test needs no +1)
                        nc.scalar.activation(
                            evac[:, :cn], ps[:, :cn],
                            mybir.ActivationFunctionType.Copy, bias=-1.0,
                        )
                        st = nc.sync.dma_start(
                            xcT[m, :, c0:c0 + cn], evac[:, :cn]
                        )
                        ph1_stores[n][m] = st

            _ph1 = ph1_gen()

            def emit_ph1(k):
                """Emit up to k phase-1 matmuls; return their instructions."""
                mms = []
                for _ in range(k):
                    mm = next(_ph1, None)
                    if mm is None:
                        break
                    mms.append(mm)
                return mms

            # prologue: two column blocks so the scan can start; the rest
            # is emitted pinned into per-step PE idle windows below
            emit_ph1(2 * MC * NKI * 2)

            # ---- phase 2: sequential LIF scan ----
            spk = [spool.tile([128, JC, BL], bf16, tag=f"spk{i}", name=f"spk{i}") for i in range(2)]
            mr = [spool.tile([128, JC, BL], f32, tag=f"mr{i}", name=f"mr{i}") for i in range(2)]
            nm = [spool.tile([128, JC, BL], f32, tag=f"nm{i}", name=f"nm{i}") for i in range(2)]
            mem = [spool.tile([128, JC, BL], f32, tag=f"mem{i}", name=f"mem{i}") for i in range(2)]
            outb = spool.tile([NS, T_STEPS, BL], f32, tag="outb")

            nc.vector.memset(spk[1][:], 0.0)
            nc.vector.memset(mr[1][:], 0.0)

            pend_ph1 = []

            for tb in range(n_tb):
                t0 = tb * CHUNK_T
                tn = min(CHUNK_T, T_STEPS - t0)
                xc_sb = ring.tile([128, MC, CHUNK_T * BL], f32, tag="xcring")
                # per-m-chunk DMAs: APs mirror the phase-1 store slices so
                # the DRAM dependency tracker reliably orders read-after-write
                for m in range(MC):
                    pref = nc.sync.dma_start(
                        xc_sb[:, m, :tn * BL],
                        xcT[m, :, t0 * BL:(t0 + tn) * BL],
                    )
                    # DRAM-tile RAW tracking misses these; order explicitly
                    st = ph1_stores[tb][m]
                    if st is not None:
                        add_dep_helper(
                            pref.ins, st.ins, reason="xc ring after ph1 store"
                        )
                for tt in range(tn):
                    t = t0 + tt
                    p, q = t % 2, (t + 1) % 2
                    # Three PSUM banks by output chunk. The matmul stream is
                    # emitted j-block-major so the NEXT step's stream (which
                    # consumes spike chunks in the same block order) never
                    # stalls: its first matmuls need only chunks 0-1, which
                    # the tiny first compare publishes right after this
                    # stream's end-of-stream semaphore.
                    SPLITS = ((0, 2), (2, 5), (5, 8))
                    pss = [
                        scps.tile([128, hi - lo, BL], f32, tag=f"scan{i}",
                                  name=f"scanps{i}")
                        for i, (lo, hi) in enumerate(SPLITS)
                    ]

                    def bank(m):
                        i = 0 if m < 2 else (1 if m < 5 else 2)
                        return pss[i][:, m - SPLITS[i][0], :]

                    xct = xc_sb[:, :, tt * BL:(tt + 1) * BL]  # holds xc - 1
                    # neg = -decay*mr_prev - (xc-1) = 1 - decay*mr - xc
                    # (runs during the matmuls; spike test becomes ps >= neg)
                    nc.vector.scalar_tensor_tensor(
                        nm[p][:], mr[q][:], -DECAY, xct, Alu.mult, Alu.subtract
                    )
                    # one accumulation group per bank: start on the bank's
                    # first matmul (clears has_written for the whole bank;
                    # later first-writes to other offsets overwrite), stop
                    # on its last
                    first_mm = None
                    last_mm = None
                    for jlo, jhi in SPLITS:      # j-blocks
                        for m in range(MC):
                            dst = bank(m)
                            blo, bhi = SPLITS[0 if m < 2 else (1 if m < 5 else 2)]
                            for j in range(jlo, jhi):
                                mm = nc.tensor.matmul(
                                    dst,
                                    rec_sb[:, j, m * 128:(m + 1) * 128],
                                    spk[q][:, j, :],
                                    start=(j == 0 and m == blo),
                                    stop=(j == JC - 1 and m == bhi - 1),
                                )
                                if first_mm is None:
                                    first_mm = mm
                                last_mm = mm
                    # pin pending phase-1 matmuls into the PE idle window
                    # between the previous step's stream and this one
                    for pm in pend_ph1:
                        add_dep_helper(
                            first_mm.ins, pm.ins, sync=False,
                            reason="ph1 mm before next scan stream",
                        )
                    pend_ph1 = emit_ph1(4)
                    for pm in pend_ph1:
                        add_dep_helper(
                            pm.ins, last_mm.ins, sync=False,
                            reason="ph1 mm after scan stream",
                        )
                    # spk = (cur_rec >= neg), smallest chunk set first
                    for i, (lo, hi) in enumerate(SPLITS):
                        nc.vector.tensor_tensor(
                            spk[p][:, lo:hi, :],
                            pss[i][:],
                            nm[p][:, lo:hi, :],
                            Alu.is_ge,
                        )
                    # mem = (ps + 1) - neg   (off critical path)
                    for i, (lo, hi) in enumerate(SPLITS):
                        nc.vector.scalar_tensor_tensor(
                            mem[p][:, lo:hi, :], pss[i][:], 1.0,
                            nm[p][:, lo:hi, :], Alu.add, Alu.subtract,
                        )
                    # mr = (mem < 1.0) * mem  == mem * (1 - spk)
                    nc.vector.scalar_tensor_tensor(
                        mr[p][:], mem[p][:], V_TH, mem[p][:], Alu.is_lt, Alu.mult
                    )
                    # output slice: first 102 hidden units live in chunk 0
                    nc.any.tensor_copy(outb[:, t, :], spk[p][0:NS, 0, :])

            nc.sync.dma_start(
                out.ap().rearrange("h (t b) -> h t b", t=T_STEPS), outb[:]
            )

    nc.compile()
    return nc


def _get_program():
    if "nc" not in _CACHE:
        _CACHE["nc"] = _build_program()
    return _CACHE["nc"]


def kernel(x: np.ndarray, fc1: np.ndarray, recurrent: np.ndarray):
    from concourse.bass_utils import run_bass_kernel_spmd

    nc = _get_program()

    x = np.asarray(x, dtype=np.float32)
    fc1_b = np.ascontiguousarray(np.asarray(fc1, np.float32)).astype(
        ml_dtypes.bfloat16
    )
    rec_b = np.ascontiguousarray(np.asarray(recurrent, np.float32)).astype(
        ml_dtypes.bfloat16
    )

    in_maps = []
    for c in range(N_CORES):
        xs = x[c * BL:(c + 1) * BL, :T_STEPS]          # [16, T, 784]
        xT_c = np.ascontiguousarray(xs.transpose(2, 1, 0).reshape(N_IN, NCOLS))
        in_maps.append(
            {"xT": xT_c.astype(ml_dtypes.bfloat16), "fc1": fc1_b, "rec": rec_b}
        )

    res = run_bass_kernel_spmd(nc, in_maps, list(range(N_CORES)))

    full = np.empty((B_TOTAL, T_STEPS, NS), dtype=np.float32)
    for c in range(N_CORES):
        o = np.asarray(res.results[c]["out"], dtype=np.float32)  # [102, T*16]
        full[c * BL:(c + 1) * BL] = o.reshape(NS, T_STEPS, BL).transpose(2, 1, 0)
    return full, T_STEPS
